# revision 17
# baseline (speedup 1.0000x reference)
"""Trainium2 Bass kernel for CrossMultiHeadedSelfAttention.

Problem: B=2, SQ=SK=2048, D=1024, H=16, HD=64 cross-attention
  q = x @ Wq + bq ; k = enc @ Wk + bk ; v = enc @ Wv + bv   (per head)
  out = softmax(q k^T / sqrt(HD)) v  -> concat heads -> @ Wo + bo

Sharding: 8 cores = 2 batches x 4 head-groups (4 heads per core).
Each core computes a partial output projection over its 4 heads; the host
sums the 4 partials per batch and adds the constant term
(bo + sum_h bv_h @ Wo_h, exact because softmax rows sum to 1).

Device-side math (per core, bf16 matmuls, f32 accumulation):
  - x/enc are pre-transposed AND pre-cast to bf16 on the host, so xT/encT
    d-major tiles load with fully contiguous DMA
  - qT/kT in [head-pair e (128) x seq] layout, bias via per-partition
    tensor_scalar add; v in natural [s, 4*65] layout with a ones column
    per head (gives softmax row-sums for free in the attn@v matmul)
  - scoresT chunk = kT_h.T @ qT_h  -> exp (scale=1/8, no max subtraction:
    scores ~ N(0,1), |s|max ~ 6 so exp is safe in f32/bf16)
  - outU = v'_h.T @ expT  ([65 x 512] in PSUM, row 64 = softmax row-sum)
  - normalize without any transpose: reciprocal of row 64 -> tiny
    partition-broadcast DMA [1,512]->[64,512] -> one tensor_mul writes the
    normalized e-major tile into the pair-stacked stk buffer
  - y = sum_pairs stk_pair.T @ Wo_pair  (K=128), DMA partial to DRAM
"""

import sys

for _p in ("/opt/trn_rl_repo", "/root/.axon_site/_ro/trn_rl_repo"):
    if _p not in sys.path:
        sys.path.insert(0, _p)

import numpy as np
import ml_dtypes

import concourse.bass as bass
import concourse.tile as tile
from concourse import bacc, mybir
from concourse.bass_utils import run_bass_kernel_spmd

BF16 = mybir.dt.bfloat16
F32 = mybir.dt.float32
AF = mybir.ActivationFunctionType

B, S, D, H, HD = 2, 2048, 1024, 16, 64
NCORES = 8
HPC = 4          # heads per core
NPAIR = 2        # head pairs per core
DC = D // 128    # 8 d-chunks
KC = S // 128    # 16 k-chunks
NQB = 4          # q blocks of 512
QB = 512
NQT = QB // 128  # q tiles per block

_CACHE = {}


def _build_program():
    nc = bacc.Bacc("TRN2", target_bir_lowering=False, debug=False, num_devices=NCORES)

    xt = nc.dram_tensor("xt", [D, S], BF16, kind="ExternalInput").ap()
    et = nc.dram_tensor("et", [D, S], BF16, kind="ExternalInput").ap()
    wq = nc.dram_tensor("wq", [128, NPAIR, DC, 128], BF16, kind="ExternalInput").ap()
    wk = nc.dram_tensor("wk", [128, NPAIR, DC, 128], BF16, kind="ExternalInput").ap()
    wv = nc.dram_tensor("wv", [128, DC, 256], BF16, kind="ExternalInput").ap()
    wo = nc.dram_tensor("wo", [128, NPAIR, D], BF16, kind="ExternalInput").ap()
    bq = nc.dram_tensor("bq", [128, NPAIR], F32, kind="ExternalInput").ap()
    bk = nc.dram_tensor("bk", [128, NPAIR], F32, kind="ExternalInput").ap()
    out = nc.dram_tensor("out", [S, D], F32, kind="ExternalOutput").ap()

    with tile.TileContext(nc) as tc:
        from contextlib import ExitStack

        with ExitStack() as ctx:
            wts = ctx.enter_context(tc.tile_pool(name="wts", bufs=1))
            big = ctx.enter_context(tc.tile_pool(name="big", bufs=1))

            # weights via gpsimd SWDGE; bulk activations via sync HWDGE
            wq_sb = wts.tile([128, NPAIR, DC, 128], BF16, name="wq_sb")
            wk_sb = wts.tile([128, NPAIR, DC, 128], BF16, name="wk_sb")
            wv_sb = wts.tile([128, DC, 256], BF16, name="wv_sb")
            wo_sb = wts.tile([128, NPAIR, D], BF16, name="wo_sb")
            bq_sb = wts.tile([128, NPAIR], F32, name="bq_sb")
            bk_sb = wts.tile([128, NPAIR], F32, name="bk_sb")
            for sb, dr in ((wq_sb, wq), (wk_sb, wk), (wv_sb, wv), (wo_sb, wo),
                           (bq_sb, bq), (bk_sb, bk)):
                nc.gpsimd.dma_start(sb, dr)

            xT = [big.tile([128, S], BF16, name=f"xT{d}") for d in range(DC)]
            eT = [big.tile([128, S], BF16, name=f"eT{d}") for d in range(DC)]
            for d in range(DC):
                nc.sync.dma_start(eT[d], et[d * 128:(d + 1) * 128, :])
            for d in range(DC):
                nc.sync.dma_start(xT[d], xt[d * 128:(d + 1) * 128, :])

            # ---- unified PSUM pools (8 banks total, live for whole kernel) ----
            dpool = ctx.enter_context(tc.tile_pool(name="dpool", bufs=4,
                                                   space="DRAM"))
            psc = ctx.enter_context(tc.tile_pool(name="psc", bufs=2, space="PSUM"))
            pou = ctx.enter_context(tc.tile_pool(name="pou", bufs=2, space="PSUM"))
            py = ctx.enter_context(tc.tile_pool(name="py", bufs=2, space="PSUM"))
            wk2 = ctx.enter_context(tc.tile_pool(name="wk2", bufs=2))
            expp = ctx.enter_context(tc.tile_pool(name="expp", bufs=6))

            # ---- projections; only kT[0] + v gate the first attention ----
            qT = [big.tile([128, S], BF16, name=f"qT{p}") for p in range(NPAIR)]
            kT = [big.tile([128, S], BF16, name=f"kT{p}") for p in range(NPAIR)]
            v = [big.tile([128, HPC, 65], BF16, name=f"v{s}") for s in range(KC)]

            def project_k(p):
                for sb4 in range(NQB):
                    sl = slice(sb4 * QB, (sb4 + 1) * QB)
                    pk = psc.tile([128, QB], F32, name="pk", tag="sc")
                    for d in range(DC):
                        nc.tensor.matmul(pk, wk_sb[:, p, d, :], eT[d][:, sl],
                                         start=(d == 0), stop=(d == DC - 1))
                    nc.vector.tensor_scalar_add(kT[p][:, sl], pk, bk_sb[:, p:p + 1])

            def project_q(p, qb):
                qsl = slice(qb * QB, (qb + 1) * QB)
                pq = py.tile([128, QB], F32, name="pq", tag="y")
                for d in range(DC):
                    nc.tensor.matmul(pq, wq_sb[:, p, d, :], xT[d][:, qsl],
                                     start=(d == 0), stop=(d == DC - 1))
                nc.vector.tensor_scalar_add(qT[p][:, qsl], pq, bq_sb[:, p:p + 1])

            project_k(0)
            for s in range(KC):
                pv = pou.tile([128, 256], F32, name="pv", tag="ou")
                for d in range(DC):
                    nc.tensor.matmul(pv, eT[d][:, s * 128:(s + 1) * 128],
                                     wv_sb[:, d, :],
                                     start=(d == 0), stop=(d == DC - 1))
                nc.vector.tensor_copy(
                    v[s][:, :, 0:64], pv.rearrange("p (h e) -> p h e", h=HPC))
                nc.vector.memset(v[s][:, :, 64:65], 1.0)

            # ---- attention + output projection ----
            for qb in range(NQB):
                qsl = slice(qb * QB, (qb + 1) * QB)
                stk = [wk2.tile([128, QB], BF16, name=f"stk{p}", tag=f"stk{p}",
                                bufs=2) for p in range(NPAIR)]
                for p in range(NPAIR):
                    project_q(p, qb)
                    ou = [pou.tile([65, QB], F32, name=f"ou{h2}", tag="ou")
                          for h2 in range(2)]
                    for kc in range(KC):
                        ksl = slice(kc * 128, (kc + 1) * 128)
                        sc = psc.tile([128, 2, QB], F32, name="sc", tag="sc")
                        ex = expp.tile([128, 2, QB], BF16, name="ex", tag="ex")
                        for h2 in range(2):
                            hp = slice(h2 * 64, (h2 + 1) * 64)
                            nc.tensor.matmul(sc[:, h2, :], kT[p][hp, ksl],
                                             qT[p][hp, qsl])
                        nc.scalar.activation(ex, sc, AF.Exp, scale=0.125)
                        for h2 in range(2):
                            nc.tensor.matmul(ou[h2], v[kc][:, 2 * p + h2, :],
                                             ex[:, h2, :],
                                             start=(kc == 0), stop=(kc == KC - 1))
                    for h2 in range(2):
                        # copy PSUM->SBUF promptly so the ou slot frees for the
                        # next pair; normalize off the critical path:
                        # reciprocal of rowsum row -> partition-broadcast via a
                        # DRAM bounce (step-0 partition APs are DRAM-only) ->
                        # one multiply into the pair-stacked e-major tile
                        osb = wk2.tile([65, QB], F32, name=f"osb{h2}",
                                       tag=f"osb{h2}", bufs=2)
                        nc.vector.tensor_copy(osb, ou[h2])
                        rr = wk2.tile([65, QB], F32, name="rr", tag="rr", bufs=4)
                        nc.vector.reciprocal_approx_fast(rr, osb)
                        rrd = dpool.tile([1, QB], F32, name="rrd", tag="rrd")
                        nc.gpsimd.dma_start(rrd, rr[64:65, :])
                        rb = wk2.tile([64, QB], F32, name="rb", tag="rb", bufs=4)
                        rr_bcast = bass.AP(tensor=rrd.tensor, offset=rrd.offset,
                                           ap=[[0, 64]] + list(rrd.ap[1:]))
                        nc.gpsimd.dma_start(rb, rr_bcast)
                        nc.vector.tensor_mul(stk[p][h2 * 64:(h2 + 1) * 64, :],
                                             osb[0:64, :], rb)
                    if qb == 0 and p == 0:
                        # kT[1] is first needed by (qb0, pair1); emitting it
                        # after (qb0, pair0) lets the first attention start
                        # as soon as kT[0] + v are ready
                        project_k(1)
                # final projection for this q block
                for qt in range(NQT):
                    tsl = slice(qt * 128, (qt + 1) * 128)
                    ysb = wk2.tile([128, D], F32, name="ysb", tag="ysb", bufs=2)
                    for dc2 in range(2):
                        dsl = slice(dc2 * 512, (dc2 + 1) * 512)
                        yp = py.tile([128, 512], F32, name="yp", tag="y")
                        for p in range(NPAIR):
                            nc.tensor.matmul(yp, stk[p][:, tsl], wo_sb[:, p, dsl],
                                             start=(p == 0), stop=(p == NPAIR - 1))
                        nc.vector.tensor_copy(ysb[:, dsl], yp)
                    nc.sync.dma_start(out[qb * QB + qt * 128:
                                          qb * QB + (qt + 1) * 128, :], ysb)

    nc.compile()
    return nc


def _bf16(a):
    return np.ascontiguousarray(a.astype(ml_dtypes.bfloat16))


def _host_prep(inputs):
    x = np.asarray(inputs["x"], np.float32)
    enc = np.asarray(inputs["encoder_output"], np.float32)
    Wq = np.asarray(inputs["Wq"], np.float32)
    bq = np.asarray(inputs["bq"], np.float32)
    Wk = np.asarray(inputs["Wk"], np.float32)
    bk = np.asarray(inputs["bk"], np.float32)
    Wv = np.asarray(inputs["Wv"], np.float32)
    Wo = np.asarray(inputs["Wo"], np.float32)

    xt_b = [_bf16(x[b].T) for b in range(B)]
    et_b = [_bf16(enc[b].T) for b in range(B)]

    in_maps = []
    for c in range(NCORES):
        b = c // 4
        hb = HPC * (c % 4)

        wq_c = Wq[hb:hb + 4].reshape(2, 2, DC, 128, HD)  # [pair, hw, dc, dp, e]
        wq_c = wq_c.transpose(3, 0, 2, 1, 4).reshape(128, NPAIR, DC, 128)
        wk_c = Wk[hb:hb + 4].reshape(2, 2, DC, 128, HD)
        wk_c = wk_c.transpose(3, 0, 2, 1, 4).reshape(128, NPAIR, DC, 128)
        wv_c = Wv[hb:hb + 4].reshape(4, DC, 128, HD)
        wv_c = wv_c.transpose(2, 1, 0, 3).reshape(128, DC, 256)
        wo_c = Wo[hb * HD:(hb + 4) * HD].reshape(2, 2, HD, D)  # [pair, hw, e, d]
        wo_c = wo_c.transpose(1, 2, 0, 3).reshape(128, NPAIR, D)
        bq_c = bq[hb:hb + 4].reshape(2, 2, HD).transpose(1, 2, 0).reshape(128, NPAIR)
        bk_c = bk[hb:hb + 4].reshape(2, 2, HD).transpose(1, 2, 0).reshape(128, NPAIR)

        in_maps.append({
            "xt": xt_b[b],
            "et": et_b[b],
            "wq": _bf16(wq_c),
            "wk": _bf16(wk_c),
            "wv": _bf16(wv_c),
            "wo": _bf16(wo_c),
            "bq": np.ascontiguousarray(bq_c),
            "bk": np.ascontiguousarray(bk_c),
        })
    return in_maps


def kernel(**inputs):
    if "nc" not in _CACHE:
        _CACHE["nc"] = _build_program()
    nc = _CACHE["nc"]

    in_maps = _host_prep(inputs)
    res = None
    for attempt in range(3):
        try:
            res = run_bass_kernel_spmd(nc, in_maps, core_ids=list(range(NCORES)))
            break
        except Exception:
            if attempt == 2:
                raise
            import time
            time.sleep(5)
    _CACHE["last_results"] = res

    bv = np.asarray(inputs["bv"], np.float32)
    Wo = np.asarray(inputs["Wo"], np.float32)
    bo = np.asarray(inputs["bo"], np.float32)
    const_d = bo + np.einsum("he,hed->d", bv,
                             Wo.reshape(H, HD, D)).astype(np.float32)

    out = np.empty((B, S, D), np.float32)
    for b in range(B):
        acc = res.results[4 * b]["out"].astype(np.float32).copy()
        for c in range(4 * b + 1, 4 * b + 4):
            acc += res.results[c]["out"]
        out[b] = acc + const_d
    return out


# revision 19
# speedup vs baseline: 1.1931x; 1.1931x over previous
"""Trainium2 Bass kernel for CrossMultiHeadedSelfAttention.

Problem: B=2, SQ=SK=2048, D=1024, H=16, HD=64 cross-attention
  q = x @ Wq + bq ; k = enc @ Wk + bk ; v = enc @ Wv + bv   (per head)
  out = softmax(q k^T / sqrt(HD)) v  -> concat heads -> @ Wo + bo

Sharding: 8 cores = 2 batches x 4 head-groups (4 heads per core).
Each core computes a partial output projection over its 4 heads; the host
sums the 4 partials per batch and adds the constant term
(bo + sum_h bv_h @ Wo_h, exact because softmax rows sum to 1).

Device-side math (per core, bf16 matmuls, f32 accumulation):
  - x/enc are pre-transposed AND pre-cast to bf16 on the host, so xT/encT
    d-major tiles load with fully contiguous DMA
  - qT/kT in [head-pair e (128) x seq] layout, bias via per-partition
    tensor_scalar add; v in natural [s, 4*65] layout with a ones column
    per head (gives softmax row-sums for free in the attn@v matmul)
  - scoresT chunk = kT_h.T @ qT_h  -> exp (scale=1/8, no max subtraction:
    scores ~ N(0,1), |s|max ~ 6 so exp is safe in f32/bf16)
  - outU = v'_h.T @ expT  ([65 x 512] in PSUM, row 64 = softmax row-sum)
  - normalize without any transpose: reciprocal of row 64 -> tiny
    partition-broadcast DMA [1,512]->[64,512] -> one tensor_mul writes the
    normalized e-major tile into the pair-stacked stk buffer
  - y = sum_pairs stk_pair.T @ Wo_pair  (K=128), DMA partial to DRAM
"""

import sys

for _p in ("/opt/trn_rl_repo", "/root/.axon_site/_ro/trn_rl_repo"):
    if _p not in sys.path:
        sys.path.insert(0, _p)

import numpy as np
import ml_dtypes

import concourse.bass as bass
import concourse.tile as tile
from concourse import bacc, mybir
from concourse.bass_utils import run_bass_kernel_spmd

BF16 = mybir.dt.bfloat16
F32 = mybir.dt.float32
AF = mybir.ActivationFunctionType

B, S, D, H, HD = 2, 2048, 1024, 16, 64
NCORES = 8
HPC = 4          # heads per core
NPAIR = 2        # head pairs per core
DC = D // 128    # 8 d-chunks
KC = S // 128    # 16 k-chunks
NQB = 4          # q blocks of 512
QB = 512
NQT = QB // 128  # q tiles per block

_CACHE = {}


def _build_program():
    nc = bacc.Bacc("TRN2", target_bir_lowering=False, debug=False, num_devices=NCORES)

    xt = nc.dram_tensor("xt", [D, S], BF16, kind="ExternalInput").ap()
    et = nc.dram_tensor("et", [D, S], BF16, kind="ExternalInput").ap()
    wq = nc.dram_tensor("wq", [128, NPAIR, DC, 128], BF16, kind="ExternalInput").ap()
    wk = nc.dram_tensor("wk", [128, NPAIR, DC, 128], BF16, kind="ExternalInput").ap()
    wv = nc.dram_tensor("wv", [128, DC, 256], BF16, kind="ExternalInput").ap()
    wo = nc.dram_tensor("wo", [128, NPAIR, D], BF16, kind="ExternalInput").ap()
    bq = nc.dram_tensor("bq", [128, NPAIR], F32, kind="ExternalInput").ap()
    bk = nc.dram_tensor("bk", [128, NPAIR], F32, kind="ExternalInput").ap()
    out = nc.dram_tensor("out", [S, D], F32, kind="ExternalOutput").ap()

    with tile.TileContext(nc) as tc:
        from contextlib import ExitStack

        with ExitStack() as ctx:
            wts = ctx.enter_context(tc.tile_pool(name="wts", bufs=1))
            big = ctx.enter_context(tc.tile_pool(name="big", bufs=1))

            # weights via gpsimd SWDGE; bulk activations via sync HWDGE
            wq_sb = wts.tile([128, NPAIR, DC, 128], BF16, name="wq_sb")
            wk_sb = wts.tile([128, NPAIR, DC, 128], BF16, name="wk_sb")
            wv_sb = wts.tile([128, DC, 256], BF16, name="wv_sb")
            wo_sb = wts.tile([128, NPAIR, D], BF16, name="wo_sb")
            bq_sb = wts.tile([128, NPAIR], F32, name="bq_sb")
            bk_sb = wts.tile([128, NPAIR], F32, name="bk_sb")
            for sb, dr in ((wq_sb, wq), (wk_sb, wk), (wv_sb, wv), (wo_sb, wo),
                           (bq_sb, bq), (bk_sb, bk)):
                nc.gpsimd.dma_start(sb, dr)

            xT = [big.tile([128, S], BF16, name=f"xT{d}") for d in range(DC)]
            eT = [big.tile([128, S], BF16, name=f"eT{d}") for d in range(DC)]
            for d in range(DC):
                nc.sync.dma_start(eT[d], et[d * 128:(d + 1) * 128, :])
            for d in range(DC):
                nc.sync.dma_start(xT[d], xt[d * 128:(d + 1) * 128, :])

            # ---- unified PSUM pools (8 banks total, live for whole kernel) ----
            dpool = ctx.enter_context(tc.tile_pool(name="dpool", bufs=4,
                                                   space="DRAM"))
            psc = ctx.enter_context(tc.tile_pool(name="psc", bufs=2, space="PSUM"))
            pou = ctx.enter_context(tc.tile_pool(name="pou", bufs=2, space="PSUM"))
            py = ctx.enter_context(tc.tile_pool(name="py", bufs=2, space="PSUM"))
            wk2 = ctx.enter_context(tc.tile_pool(name="wk2", bufs=2))
            expp = ctx.enter_context(tc.tile_pool(name="expp", bufs=6))

            # ---- projections; only kT[0] + v gate the first attention ----
            qT = [big.tile([128, S], BF16, name=f"qT{p}") for p in range(NPAIR)]
            kT = [big.tile([128, S], BF16, name=f"kT{p}") for p in range(NPAIR)]
            v = [big.tile([128, HPC, 65], BF16, name=f"v{s}") for s in range(KC)]

            def project_k(p):
                for sb4 in range(NQB):
                    sl = slice(sb4 * QB, (sb4 + 1) * QB)
                    pk = psc.tile([128, QB], F32, name="pk", tag="sc")
                    for d in range(DC):
                        nc.tensor.matmul(pk, wk_sb[:, p, d, :], eT[d][:, sl],
                                         start=(d == 0), stop=(d == DC - 1))
                    nc.vector.tensor_scalar_add(kT[p][:, sl], pk, bk_sb[:, p:p + 1])

            def project_q(p, qb):
                qsl = slice(qb * QB, (qb + 1) * QB)
                pq = psc.tile([128, QB], F32, name="pq", tag="sc")
                for d in range(DC):
                    nc.tensor.matmul(pq, wq_sb[:, p, d, :], xT[d][:, qsl],
                                     start=(d == 0), stop=(d == DC - 1))
                nc.vector.tensor_scalar_add(qT[p][:, qsl], pq, bq_sb[:, p:p + 1])

            project_k(0)
            for s in range(KC):
                pv = pou.tile([128, 256], F32, name="pv", tag="ou")
                for d in range(DC):
                    nc.tensor.matmul(pv, eT[d][:, s * 128:(s + 1) * 128],
                                     wv_sb[:, d, :],
                                     start=(d == 0), stop=(d == DC - 1))
                nc.vector.tensor_copy(
                    v[s][:, :, 0:64], pv.rearrange("p (h e) -> p h e", h=HPC))
                nc.vector.memset(v[s][:, :, 64:65], 1.0)

            # ---- attention + output projection ----
            for qb in range(NQB):
                qsl = slice(qb * QB, (qb + 1) * QB)
                for p in range(NPAIR):
                    project_q(p, qb)
                stk = [wk2.tile([128, QB], BF16, name=f"stk{p}", tag=f"stk{p}",
                                bufs=2) for p in range(NPAIR)]
                for p in range(NPAIR):
                    ou = [pou.tile([65, QB], F32, name=f"ou{h2}", tag="ou")
                          for h2 in range(2)]
                    for kc in range(KC):
                        ksl = slice(kc * 128, (kc + 1) * 128)
                        sc = psc.tile([128, 2, QB], F32, name="sc", tag="sc")
                        ex = expp.tile([128, 2, QB], BF16, name="ex", tag="ex")
                        for h2 in range(2):
                            hp = slice(h2 * 64, (h2 + 1) * 64)
                            nc.tensor.matmul(sc[:, h2, :], kT[p][hp, ksl],
                                             qT[p][hp, qsl])
                        nc.scalar.activation(ex, sc, AF.Exp, scale=0.125)
                        for h2 in range(2):
                            nc.tensor.matmul(ou[h2], v[kc][:, 2 * p + h2, :],
                                             ex[:, h2, :],
                                             start=(kc == 0), stop=(kc == KC - 1))
                    for h2 in range(2):
                        # copy PSUM->SBUF promptly so the ou slot frees for the
                        # next pair; normalize off the critical path:
                        # reciprocal of rowsum row -> partition-broadcast via a
                        # DRAM bounce (step-0 partition APs are DRAM-only) ->
                        # one multiply into the pair-stacked e-major tile
                        osb = wk2.tile([65, QB], F32, name=f"osb{h2}",
                                       tag=f"osb{h2}", bufs=2)
                        nc.vector.tensor_copy(osb, ou[h2])
                        rr = wk2.tile([65, QB], F32, name="rr", tag="rr", bufs=4)
                        nc.vector.reciprocal_approx_fast(rr, osb)
                        rrd = dpool.tile([1, QB], F32, name="rrd", tag="rrd")
                        nc.gpsimd.dma_start(rrd, rr[64:65, :])
                        rb = wk2.tile([64, QB], F32, name="rb", tag="rb", bufs=4)
                        rr_bcast = bass.AP(tensor=rrd.tensor, offset=rrd.offset,
                                           ap=[[0, 64]] + list(rrd.ap[1:]))
                        nc.gpsimd.dma_start(rb, rr_bcast)
                        nc.vector.tensor_mul(stk[p][h2 * 64:(h2 + 1) * 64, :],
                                             osb[0:64, :], rb)
                    if qb == 0 and p == 0:
                        # kT[1] is first needed by (qb0, pair1); emitting it
                        # after (qb0, pair0) lets the first attention start
                        # as soon as kT[0] + v are ready
                        project_k(1)
                # final projection for this q block
                for qt in range(NQT):
                    tsl = slice(qt * 128, (qt + 1) * 128)
                    ysb = wk2.tile([128, D], F32, name="ysb", tag="ysb", bufs=2)
                    for dc2 in range(2):
                        dsl = slice(dc2 * 512, (dc2 + 1) * 512)
                        yp = py.tile([128, 512], F32, name="yp", tag="y")
                        for p in range(NPAIR):
                            nc.tensor.matmul(yp, stk[p][:, tsl], wo_sb[:, p, dsl],
                                             start=(p == 0), stop=(p == NPAIR - 1))
                        nc.vector.tensor_copy(ysb[:, dsl], yp)
                    nc.sync.dma_start(out[qb * QB + qt * 128:
                                          qb * QB + (qt + 1) * 128, :], ysb)

    nc.compile()
    return nc


def _bf16(a):
    return np.ascontiguousarray(a.astype(ml_dtypes.bfloat16))


def _host_prep(inputs):
    x = np.asarray(inputs["x"], np.float32)
    enc = np.asarray(inputs["encoder_output"], np.float32)
    Wq = np.asarray(inputs["Wq"], np.float32)
    bq = np.asarray(inputs["bq"], np.float32)
    Wk = np.asarray(inputs["Wk"], np.float32)
    bk = np.asarray(inputs["bk"], np.float32)
    Wv = np.asarray(inputs["Wv"], np.float32)
    Wo = np.asarray(inputs["Wo"], np.float32)

    xt_b = [_bf16(x[b].T) for b in range(B)]
    et_b = [_bf16(enc[b].T) for b in range(B)]

    in_maps = []
    for c in range(NCORES):
        b = c // 4
        hb = HPC * (c % 4)

        wq_c = Wq[hb:hb + 4].reshape(2, 2, DC, 128, HD)  # [pair, hw, dc, dp, e]
        wq_c = wq_c.transpose(3, 0, 2, 1, 4).reshape(128, NPAIR, DC, 128)
        wk_c = Wk[hb:hb + 4].reshape(2, 2, DC, 128, HD)
        wk_c = wk_c.transpose(3, 0, 2, 1, 4).reshape(128, NPAIR, DC, 128)
        wv_c = Wv[hb:hb + 4].reshape(4, DC, 128, HD)
        wv_c = wv_c.transpose(2, 1, 0, 3).reshape(128, DC, 256)
        wo_c = Wo[hb * HD:(hb + 4) * HD].reshape(2, 2, HD, D)  # [pair, hw, e, d]
        wo_c = wo_c.transpose(1, 2, 0, 3).reshape(128, NPAIR, D)
        bq_c = bq[hb:hb + 4].reshape(2, 2, HD).transpose(1, 2, 0).reshape(128, NPAIR)
        bk_c = bk[hb:hb + 4].reshape(2, 2, HD).transpose(1, 2, 0).reshape(128, NPAIR)

        in_maps.append({
            "xt": xt_b[b],
            "et": et_b[b],
            "wq": _bf16(wq_c),
            "wk": _bf16(wk_c),
            "wv": _bf16(wv_c),
            "wo": _bf16(wo_c),
            "bq": np.ascontiguousarray(bq_c),
            "bk": np.ascontiguousarray(bk_c),
        })
    return in_maps


def kernel(**inputs):
    if "nc" not in _CACHE:
        _CACHE["nc"] = _build_program()
    nc = _CACHE["nc"]

    in_maps = _host_prep(inputs)
    res = None
    for attempt in range(3):
        try:
            res = run_bass_kernel_spmd(nc, in_maps, core_ids=list(range(NCORES)))
            break
        except Exception:
            if attempt == 2:
                raise
            import time
            time.sleep(5)
    _CACHE["last_results"] = res

    bv = np.asarray(inputs["bv"], np.float32)
    Wo = np.asarray(inputs["Wo"], np.float32)
    bo = np.asarray(inputs["bo"], np.float32)
    const_d = bo + np.einsum("he,hed->d", bv,
                             Wo.reshape(H, HD, D)).astype(np.float32)

    out = np.empty((B, S, D), np.float32)
    for b in range(B):
        acc = res.results[4 * b]["out"].astype(np.float32).copy()
        for c in range(4 * b + 1, 4 * b + 4):
            acc += res.results[c]["out"]
        out[b] = acc + const_d
    return out


# revision 22
# speedup vs baseline: 1.1974x; 1.0036x over previous
"""Trainium2 Bass kernel for CrossMultiHeadedSelfAttention.

Problem: B=2, SQ=SK=2048, D=1024, H=16, HD=64 cross-attention
  q = x @ Wq + bq ; k = enc @ Wk + bk ; v = enc @ Wv + bv   (per head)
  out = softmax(q k^T / sqrt(HD)) v  -> concat heads -> @ Wo + bo

Sharding: 8 cores = 2 batches x 4 head-groups (4 heads per core).
Each core computes a partial output projection over its 4 heads; the host
sums the 4 partials per batch and adds the constant term
(bo + sum_h bv_h @ Wo_h, exact because softmax rows sum to 1).

Device-side math (per core, bf16 matmuls, f32 accumulation):
  - x/enc are pre-transposed AND pre-cast to bf16 on the host, so xT/encT
    d-major tiles load with fully contiguous DMA
  - qT/kT in [head-pair e (128) x seq] layout, bias via per-partition
    tensor_scalar add; v in natural [s, 4*65] layout with a ones column
    per head (gives softmax row-sums for free in the attn@v matmul)
  - scoresT chunk = kT_h.T @ qT_h  -> exp (scale=1/8, no max subtraction:
    scores ~ N(0,1), |s|max ~ 6 so exp is safe in f32/bf16)
  - outU = v'_h.T @ expT  ([65 x 512] in PSUM, row 64 = softmax row-sum)
  - normalize without any transpose: reciprocal of row 64 -> tiny
    partition-broadcast DMA [1,512]->[64,512] -> one tensor_mul writes the
    normalized e-major tile into the pair-stacked stk buffer
  - y = sum_pairs stk_pair.T @ Wo_pair  (K=128), DMA partial to DRAM
"""

import sys

for _p in ("/opt/trn_rl_repo", "/root/.axon_site/_ro/trn_rl_repo"):
    if _p not in sys.path:
        sys.path.insert(0, _p)

import numpy as np
import ml_dtypes

import concourse.bass as bass
import concourse.tile as tile
from concourse import bacc, mybir
from concourse.bass_utils import run_bass_kernel_spmd

BF16 = mybir.dt.bfloat16
F32 = mybir.dt.float32
AF = mybir.ActivationFunctionType

B, S, D, H, HD = 2, 2048, 1024, 16, 64
NCORES = 8
HPC = 4          # heads per core
NPAIR = 2        # head pairs per core
DC = D // 128    # 8 d-chunks
KC = S // 128    # 16 k-chunks
NQB = 4          # q blocks of 512
QB = 512
NQT = QB // 128  # q tiles per block

_CACHE = {}


def _build_program():
    nc = bacc.Bacc("TRN2", target_bir_lowering=False, debug=False, num_devices=NCORES)

    xt = nc.dram_tensor("xt", [D, S], BF16, kind="ExternalInput").ap()
    et = nc.dram_tensor("et", [D, S], BF16, kind="ExternalInput").ap()
    wq = nc.dram_tensor("wq", [128, NPAIR, DC, 128], BF16, kind="ExternalInput").ap()
    wk = nc.dram_tensor("wk", [128, NPAIR, DC, 128], BF16, kind="ExternalInput").ap()
    wv = nc.dram_tensor("wv", [128, DC, 256], BF16, kind="ExternalInput").ap()
    wo = nc.dram_tensor("wo", [128, NPAIR, D], BF16, kind="ExternalInput").ap()
    bq = nc.dram_tensor("bq", [128, NPAIR], F32, kind="ExternalInput").ap()
    bk = nc.dram_tensor("bk", [128, NPAIR], F32, kind="ExternalInput").ap()
    out = nc.dram_tensor("out", [S, D], F32, kind="ExternalOutput").ap()

    with tile.TileContext(nc) as tc:
        from contextlib import ExitStack

        with ExitStack() as ctx:
            wts = ctx.enter_context(tc.tile_pool(name="wts", bufs=1))
            big = ctx.enter_context(tc.tile_pool(name="big", bufs=1))

            # weights via gpsimd SWDGE; bulk activations via sync HWDGE
            wq_sb = wts.tile([128, NPAIR, DC, 128], BF16, name="wq_sb")
            wk_sb = wts.tile([128, NPAIR, DC, 128], BF16, name="wk_sb")
            wv_sb = wts.tile([128, DC, 256], BF16, name="wv_sb")
            wo_sb = wts.tile([128, NPAIR, D], BF16, name="wo_sb")
            bq_sb = wts.tile([128, NPAIR], F32, name="bq_sb")
            bk_sb = wts.tile([128, NPAIR], F32, name="bk_sb")
            for sb, dr in ((wq_sb, wq), (wk_sb, wk), (wv_sb, wv), (wo_sb, wo),
                           (bq_sb, bq), (bk_sb, bk)):
                nc.gpsimd.dma_start(sb, dr)

            xT = [big.tile([128, S], BF16, name=f"xT{d}") for d in range(DC)]
            eT = [big.tile([128, S], BF16, name=f"eT{d}") for d in range(DC)]
            for d in range(DC):
                nc.sync.dma_start(eT[d], et[d * 128:(d + 1) * 128, :])
            for d in range(DC):
                nc.sync.dma_start(xT[d], xt[d * 128:(d + 1) * 128, :])

            # ---- unified PSUM pools (8 banks total, live for whole kernel) ----
            dpool = ctx.enter_context(tc.tile_pool(name="dpool", bufs=4,
                                                   space="DRAM"))
            psc = ctx.enter_context(tc.tile_pool(name="psc", bufs=2, space="PSUM"))
            pou = ctx.enter_context(tc.tile_pool(name="pou", bufs=2, space="PSUM"))
            py = ctx.enter_context(tc.tile_pool(name="py", bufs=2, space="PSUM"))
            wk2 = ctx.enter_context(tc.tile_pool(name="wk2", bufs=2))
            expp = ctx.enter_context(tc.tile_pool(name="expp", bufs=6))

            # ---- projections; only kT[0] + v gate the first attention ----
            qT = [big.tile([128, S], BF16, name=f"qT{p}") for p in range(NPAIR)]
            kT = [big.tile([128, S], BF16, name=f"kT{p}") for p in range(NPAIR)]
            v = [big.tile([128, HPC, 65], BF16, name=f"v{s}") for s in range(KC)]

            def project_k_chunk(p, sb4):
                # deferred projections use the y-slot (idle during attention)
                sl = slice(sb4 * QB, (sb4 + 1) * QB)
                pk = py.tile([128, QB], F32, name="pk", tag="y")
                for d in range(DC):
                    nc.tensor.matmul(pk, wk_sb[:, p, d, :], eT[d][:, sl],
                                     start=(d == 0), stop=(d == DC - 1))
                nc.vector.tensor_scalar_add(kT[p][:, sl], pk, bk_sb[:, p:p + 1])

            def project_q(p, qb):
                qsl = slice(qb * QB, (qb + 1) * QB)
                pq = psc.tile([128, QB], F32, name="pq", tag="sc")
                for d in range(DC):
                    nc.tensor.matmul(pq, wq_sb[:, p, d, :], xT[d][:, qsl],
                                     start=(d == 0), stop=(d == DC - 1))
                nc.vector.tensor_scalar_add(qT[p][:, qsl], pq, bq_sb[:, p:p + 1])

            def project_v_tile(s):
                pv = py.tile([128, 256], F32, name="pv", tag="y")
                for d in range(DC):
                    nc.tensor.matmul(pv, eT[d][:, s * 128:(s + 1) * 128],
                                     wv_sb[:, d, :],
                                     start=(d == 0), stop=(d == DC - 1))
                nc.vector.tensor_copy(
                    v[s][:, :, 0:64], pv.rearrange("p (h e) -> p h e", h=HPC))
                nc.vector.memset(v[s][:, :, 64:65], 1.0)

            # minimal prologue: just what (qb0, pair0, kc0..3) needs;
            # the rest of the projections are interjected into the first
            # q block's attention loops (PE slack there, ACT is the
            # bottleneck engine inside the kc loop)
            project_k_chunk(0, 0)
            for s in range(4):
                project_v_tile(s)
            project_q(0, 0)

            def interject_qb0_p0(kc):
                if kc == 0:
                    project_k_chunk(0, 1)
                elif kc == 1:
                    project_v_tile(4); project_v_tile(5)
                elif kc == 2:
                    project_v_tile(6); project_v_tile(7)
                elif kc == 4:
                    project_k_chunk(0, 2)
                elif kc == 5:
                    project_v_tile(8); project_v_tile(9)
                elif kc == 6:
                    project_v_tile(10); project_v_tile(11)
                elif kc == 8:
                    project_k_chunk(0, 3)
                elif kc == 9:
                    project_v_tile(12); project_v_tile(13)
                elif kc == 10:
                    project_v_tile(14); project_v_tile(15)
                elif kc == 12:
                    project_k_chunk(1, 0)
                elif kc == 13:
                    project_q(1, 0)

            def interject_qb0_p1(kc):
                if kc == 0:
                    project_k_chunk(1, 1)
                elif kc == 4:
                    project_k_chunk(1, 2)
                elif kc == 8:
                    project_k_chunk(1, 3)

            # ---- attention + output projection ----
            for qb in range(NQB):
                qsl = slice(qb * QB, (qb + 1) * QB)
                if qb > 0:
                    for p in range(NPAIR):
                        project_q(p, qb)
                stk = [wk2.tile([128, QB], BF16, name=f"stk{p}", tag=f"stk{p}",
                                bufs=2) for p in range(NPAIR)]
                for p in range(NPAIR):
                    if qb == 0:
                        interject = interject_qb0_p0 if p == 0 else interject_qb0_p1
                    else:
                        interject = None
                    ou = [pou.tile([65, QB], F32, name=f"ou{h2}", tag="ou")
                          for h2 in range(2)]
                    for kc in range(KC):
                        ksl = slice(kc * 128, (kc + 1) * 128)
                        sc = psc.tile([128, 2, QB], F32, name="sc", tag="sc")
                        ex = expp.tile([128, 2, QB], BF16, name="ex", tag="ex")
                        for h2 in range(2):
                            hp = slice(h2 * 64, (h2 + 1) * 64)
                            nc.tensor.matmul(sc[:, h2, :], kT[p][hp, ksl],
                                             qT[p][hp, qsl])
                        nc.scalar.activation(ex, sc, AF.Exp, scale=0.125)
                        for h2 in range(2):
                            nc.tensor.matmul(ou[h2], v[kc][:, 2 * p + h2, :],
                                             ex[:, h2, :],
                                             start=(kc == 0), stop=(kc == KC - 1))
                        if interject is not None:
                            interject(kc)
                    for h2 in range(2):
                        # copy PSUM->SBUF promptly so the ou slot frees for the
                        # next pair; normalize off the critical path:
                        # reciprocal of rowsum row -> partition-broadcast via a
                        # DRAM bounce (step-0 partition APs are DRAM-only) ->
                        # one multiply into the pair-stacked e-major tile
                        osb = wk2.tile([65, QB], F32, name=f"osb{h2}",
                                       tag=f"osb{h2}", bufs=2)
                        nc.vector.tensor_copy(osb, ou[h2])
                        rr = wk2.tile([65, QB], F32, name="rr", tag="rr", bufs=4)
                        nc.vector.reciprocal_approx_fast(rr, osb)
                        rrd = dpool.tile([1, QB], F32, name="rrd", tag="rrd")
                        nc.gpsimd.dma_start(rrd, rr[64:65, :])
                        rb = wk2.tile([64, QB], F32, name="rb", tag="rb", bufs=4)
                        rr_bcast = bass.AP(tensor=rrd.tensor, offset=rrd.offset,
                                           ap=[[0, 64]] + list(rrd.ap[1:]))
                        nc.gpsimd.dma_start(rb, rr_bcast)
                        nc.vector.tensor_mul(stk[p][h2 * 64:(h2 + 1) * 64, :],
                                             osb[0:64, :], rb)
                # final projection for this q block
                for qt in range(NQT):
                    tsl = slice(qt * 128, (qt + 1) * 128)
                    ysb = wk2.tile([128, D], F32, name="ysb", tag="ysb", bufs=2)
                    for dc2 in range(2):
                        dsl = slice(dc2 * 512, (dc2 + 1) * 512)
                        yp = py.tile([128, 512], F32, name="yp", tag="y")
                        for p in range(NPAIR):
                            nc.tensor.matmul(yp, stk[p][:, tsl], wo_sb[:, p, dsl],
                                             start=(p == 0), stop=(p == NPAIR - 1))
                        nc.vector.tensor_copy(ysb[:, dsl], yp)
                    nc.sync.dma_start(out[qb * QB + qt * 128:
                                          qb * QB + (qt + 1) * 128, :], ysb)

    nc.compile()
    return nc


def _bf16(a):
    return np.ascontiguousarray(a.astype(ml_dtypes.bfloat16))


def _host_prep(inputs):
    x = np.asarray(inputs["x"], np.float32)
    enc = np.asarray(inputs["encoder_output"], np.float32)
    Wq = np.asarray(inputs["Wq"], np.float32)
    bq = np.asarray(inputs["bq"], np.float32)
    Wk = np.asarray(inputs["Wk"], np.float32)
    bk = np.asarray(inputs["bk"], np.float32)
    Wv = np.asarray(inputs["Wv"], np.float32)
    Wo = np.asarray(inputs["Wo"], np.float32)

    xt_b = [_bf16(x[b].T) for b in range(B)]
    et_b = [_bf16(enc[b].T) for b in range(B)]

    in_maps = []
    for c in range(NCORES):
        b = c // 4
        hb = HPC * (c % 4)

        wq_c = Wq[hb:hb + 4].reshape(2, 2, DC, 128, HD)  # [pair, hw, dc, dp, e]
        wq_c = wq_c.transpose(3, 0, 2, 1, 4).reshape(128, NPAIR, DC, 128)
        wk_c = Wk[hb:hb + 4].reshape(2, 2, DC, 128, HD)
        wk_c = wk_c.transpose(3, 0, 2, 1, 4).reshape(128, NPAIR, DC, 128)
        wv_c = Wv[hb:hb + 4].reshape(4, DC, 128, HD)
        wv_c = wv_c.transpose(2, 1, 0, 3).reshape(128, DC, 256)
        wo_c = Wo[hb * HD:(hb + 4) * HD].reshape(2, 2, HD, D)  # [pair, hw, e, d]
        wo_c = wo_c.transpose(1, 2, 0, 3).reshape(128, NPAIR, D)
        bq_c = bq[hb:hb + 4].reshape(2, 2, HD).transpose(1, 2, 0).reshape(128, NPAIR)
        bk_c = bk[hb:hb + 4].reshape(2, 2, HD).transpose(1, 2, 0).reshape(128, NPAIR)

        in_maps.append({
            "xt": xt_b[b],
            "et": et_b[b],
            "wq": _bf16(wq_c),
            "wk": _bf16(wk_c),
            "wv": _bf16(wv_c),
            "wo": _bf16(wo_c),
            "bq": np.ascontiguousarray(bq_c),
            "bk": np.ascontiguousarray(bk_c),
        })
    return in_maps


def kernel(**inputs):
    if "nc" not in _CACHE:
        _CACHE["nc"] = _build_program()
    nc = _CACHE["nc"]

    in_maps = _host_prep(inputs)
    res = None
    for attempt in range(3):
        try:
            res = run_bass_kernel_spmd(nc, in_maps, core_ids=list(range(NCORES)))
            break
        except Exception:
            if attempt == 2:
                raise
            import time
            time.sleep(5)
    _CACHE["last_results"] = res

    bv = np.asarray(inputs["bv"], np.float32)
    Wo = np.asarray(inputs["Wo"], np.float32)
    bo = np.asarray(inputs["bo"], np.float32)
    const_d = bo + np.einsum("he,hed->d", bv,
                             Wo.reshape(H, HD, D)).astype(np.float32)

    out = np.empty((B, S, D), np.float32)
    for b in range(B):
        acc = res.results[4 * b]["out"].astype(np.float32).copy()
        for c in range(4 * b + 1, 4 * b + 4):
            acc += res.results[c]["out"]
        out[b] = acc + const_d
    return out


# revision 23
# speedup vs baseline: 1.2336x; 1.0303x over previous
"""Trainium2 Bass kernel for CrossMultiHeadedSelfAttention.

Problem: B=2, SQ=SK=2048, D=1024, H=16, HD=64 cross-attention
  q = x @ Wq + bq ; k = enc @ Wk + bk ; v = enc @ Wv + bv   (per head)
  out = softmax(q k^T / sqrt(HD)) v  -> concat heads -> @ Wo + bo

Sharding: 8 cores = 2 batches x 4 head-groups (4 heads per core).
Each core computes a partial output projection over its 4 heads; the host
sums the 4 partials per batch and adds the constant term
(bo + sum_h bv_h @ Wo_h, exact because softmax rows sum to 1).

Device-side math (per core, bf16 matmuls, f32 accumulation):
  - x/enc are pre-transposed AND pre-cast to bf16 on the host, so xT/encT
    d-major tiles load with fully contiguous DMA
  - qT/kT in [head-pair e (128) x seq] layout, bias via per-partition
    tensor_scalar add; v in natural [s, 4*65] layout with a ones column
    per head (gives softmax row-sums for free in the attn@v matmul)
  - scoresT chunk = kT_h.T @ qT_h  -> exp (scale=1/8, no max subtraction:
    scores ~ N(0,1), |s|max ~ 6 so exp is safe in f32/bf16)
  - outU = v'_h.T @ expT  ([65 x 512] in PSUM, row 64 = softmax row-sum)
  - normalize without any transpose: reciprocal of row 64 -> tiny
    partition-broadcast DMA [1,512]->[64,512] -> one tensor_mul writes the
    normalized e-major tile into the pair-stacked stk buffer
  - y = sum_pairs stk_pair.T @ Wo_pair  (K=128), DMA partial to DRAM
"""

import sys

for _p in ("/opt/trn_rl_repo", "/root/.axon_site/_ro/trn_rl_repo"):
    if _p not in sys.path:
        sys.path.insert(0, _p)

import numpy as np
import ml_dtypes

import concourse.bass as bass
import concourse.tile as tile
from concourse import bacc, mybir
from concourse.bass_utils import run_bass_kernel_spmd

BF16 = mybir.dt.bfloat16
F32 = mybir.dt.float32
AF = mybir.ActivationFunctionType

B, S, D, H, HD = 2, 2048, 1024, 16, 64
NCORES = 8
HPC = 4          # heads per core
NPAIR = 2        # head pairs per core
DC = D // 128    # 8 d-chunks
KC = S // 128    # 16 k-chunks
NQB = 4          # q blocks of 512
QB = 512
NQT = QB // 128  # q tiles per block

_CACHE = {}


def _build_program():
    nc = bacc.Bacc("TRN2", target_bir_lowering=False, debug=False, num_devices=NCORES)

    xt = nc.dram_tensor("xt", [D, S], BF16, kind="ExternalInput").ap()
    et = nc.dram_tensor("et", [D, S], BF16, kind="ExternalInput").ap()
    wq = nc.dram_tensor("wq", [128, NPAIR, DC, 128], BF16, kind="ExternalInput").ap()
    wk = nc.dram_tensor("wk", [128, NPAIR, DC, 128], BF16, kind="ExternalInput").ap()
    wv = nc.dram_tensor("wv", [128, DC, 256], BF16, kind="ExternalInput").ap()
    wo = nc.dram_tensor("wo", [128, NPAIR, D], BF16, kind="ExternalInput").ap()
    bq = nc.dram_tensor("bq", [128, NPAIR], F32, kind="ExternalInput").ap()
    bk = nc.dram_tensor("bk", [128, NPAIR], F32, kind="ExternalInput").ap()
    out = nc.dram_tensor("out", [S, D], F32, kind="ExternalOutput").ap()

    with tile.TileContext(nc) as tc:
        from contextlib import ExitStack

        with ExitStack() as ctx:
            wts = ctx.enter_context(tc.tile_pool(name="wts", bufs=1))
            big = ctx.enter_context(tc.tile_pool(name="big", bufs=1))

            # weights via gpsimd SWDGE; bulk activations via sync HWDGE
            wq_sb = wts.tile([128, NPAIR, DC, 128], BF16, name="wq_sb")
            wk_sb = wts.tile([128, NPAIR, DC, 128], BF16, name="wk_sb")
            wv_sb = wts.tile([128, DC, 256], BF16, name="wv_sb")
            wo_sb = wts.tile([128, NPAIR, D], BF16, name="wo_sb")
            bq_sb = wts.tile([128, NPAIR], F32, name="bq_sb")
            bk_sb = wts.tile([128, NPAIR], F32, name="bk_sb")
            for sb, dr in ((wq_sb, wq), (wk_sb, wk), (wv_sb, wv), (wo_sb, wo),
                           (bq_sb, bq), (bk_sb, bk)):
                nc.gpsimd.dma_start(sb, dr)

            # column-block loads so the first projection chunk only waits on
            # ~1MB of activations, not the full 8MB
            xT = [big.tile([128, S], BF16, name=f"xT{d}") for d in range(DC)]
            eT = [big.tile([128, S], BF16, name=f"eT{d}") for d in range(DC)]
            for sb4 in range(NQB):
                sl = slice(sb4 * QB, (sb4 + 1) * QB)
                for d in range(DC):
                    nc.sync.dma_start(eT[d][:, sl], et[d * 128:(d + 1) * 128, sl])
                if sb4 == 0:
                    for d in range(DC):
                        nc.sync.dma_start(xT[d][:, sl],
                                          xt[d * 128:(d + 1) * 128, sl])
            for sb4 in range(1, NQB):
                sl = slice(sb4 * QB, (sb4 + 1) * QB)
                for d in range(DC):
                    nc.sync.dma_start(xT[d][:, sl], xt[d * 128:(d + 1) * 128, sl])

            # ---- unified PSUM pools (8 banks total, live for whole kernel) ----
            dpool = ctx.enter_context(tc.tile_pool(name="dpool", bufs=4,
                                                   space="DRAM"))
            psc = ctx.enter_context(tc.tile_pool(name="psc", bufs=2, space="PSUM"))
            pou = ctx.enter_context(tc.tile_pool(name="pou", bufs=2, space="PSUM"))
            py = ctx.enter_context(tc.tile_pool(name="py", bufs=2, space="PSUM"))
            wk2 = ctx.enter_context(tc.tile_pool(name="wk2", bufs=2))
            expp = ctx.enter_context(tc.tile_pool(name="expp", bufs=6))

            # ---- projections; only kT[0] + v gate the first attention ----
            qT = [big.tile([128, S], BF16, name=f"qT{p}") for p in range(NPAIR)]
            kT = [big.tile([128, S], BF16, name=f"kT{p}") for p in range(NPAIR)]
            v = [big.tile([128, HPC, 65], BF16, name=f"v{s}") for s in range(KC)]

            def project_k_chunk(p, sb4):
                # deferred projections use the y-slot (idle during attention)
                sl = slice(sb4 * QB, (sb4 + 1) * QB)
                pk = py.tile([128, QB], F32, name="pk", tag="y")
                for d in range(DC):
                    nc.tensor.matmul(pk, wk_sb[:, p, d, :], eT[d][:, sl],
                                     start=(d == 0), stop=(d == DC - 1))
                nc.vector.tensor_scalar_add(kT[p][:, sl], pk, bk_sb[:, p:p + 1])

            def project_q(p, qb):
                qsl = slice(qb * QB, (qb + 1) * QB)
                pq = psc.tile([128, QB], F32, name="pq", tag="sc")
                for d in range(DC):
                    nc.tensor.matmul(pq, wq_sb[:, p, d, :], xT[d][:, qsl],
                                     start=(d == 0), stop=(d == DC - 1))
                nc.vector.tensor_scalar_add(qT[p][:, qsl], pq, bq_sb[:, p:p + 1])

            def project_v_tile(s):
                pv = py.tile([128, 256], F32, name="pv", tag="y")
                for d in range(DC):
                    nc.tensor.matmul(pv, eT[d][:, s * 128:(s + 1) * 128],
                                     wv_sb[:, d, :],
                                     start=(d == 0), stop=(d == DC - 1))
                nc.vector.tensor_copy(
                    v[s][:, :, 0:64], pv.rearrange("p (h e) -> p h e", h=HPC))
                nc.vector.memset(v[s][:, :, 64:65], 1.0)

            # minimal prologue: just what (qb0, pair0, kc0..3) needs;
            # the rest of the projections are interjected into the first
            # q block's attention loops (PE slack there, ACT is the
            # bottleneck engine inside the kc loop)
            project_k_chunk(0, 0)
            for s in range(4):
                project_v_tile(s)
            project_q(0, 0)

            def interject_qb0_p0(kc):
                if kc == 0:
                    project_k_chunk(0, 1)
                elif kc == 1:
                    project_v_tile(4); project_v_tile(5)
                elif kc == 2:
                    project_v_tile(6); project_v_tile(7)
                elif kc == 4:
                    project_k_chunk(0, 2)
                elif kc == 5:
                    project_v_tile(8); project_v_tile(9)
                elif kc == 6:
                    project_v_tile(10); project_v_tile(11)
                elif kc == 8:
                    project_k_chunk(0, 3)
                elif kc == 9:
                    project_v_tile(12); project_v_tile(13)
                elif kc == 10:
                    project_v_tile(14); project_v_tile(15)
                elif kc == 12:
                    project_k_chunk(1, 0)
                elif kc == 13:
                    project_q(1, 0)

            def interject_qb0_p1(kc):
                if kc == 0:
                    project_k_chunk(1, 1)
                elif kc == 4:
                    project_k_chunk(1, 2)
                elif kc == 8:
                    project_k_chunk(1, 3)

            # ---- attention + output projection ----
            for qb in range(NQB):
                qsl = slice(qb * QB, (qb + 1) * QB)
                if qb > 0:
                    for p in range(NPAIR):
                        project_q(p, qb)
                stk = [wk2.tile([128, QB], BF16, name=f"stk{p}", tag=f"stk{p}",
                                bufs=2) for p in range(NPAIR)]
                for p in range(NPAIR):
                    if qb == 0:
                        interject = interject_qb0_p0 if p == 0 else interject_qb0_p1
                    else:
                        interject = None
                    ou = [pou.tile([65, QB], F32, name=f"ou{h2}", tag="ou")
                          for h2 in range(2)]
                    for kc in range(KC):
                        ksl = slice(kc * 128, (kc + 1) * 128)
                        sc = psc.tile([128, 2, QB], F32, name="sc", tag="sc")
                        ex = expp.tile([128, 2, QB], BF16, name="ex", tag="ex")
                        for h2 in range(2):
                            hp = slice(h2 * 64, (h2 + 1) * 64)
                            nc.tensor.matmul(sc[:, h2, :], kT[p][hp, ksl],
                                             qT[p][hp, qsl])
                        nc.scalar.activation(ex, sc, AF.Exp, scale=0.125)
                        for h2 in range(2):
                            nc.tensor.matmul(ou[h2], v[kc][:, 2 * p + h2, :],
                                             ex[:, h2, :],
                                             start=(kc == 0), stop=(kc == KC - 1))
                        if interject is not None:
                            interject(kc)
                    for h2 in range(2):
                        # copy PSUM->SBUF promptly so the ou slot frees for the
                        # next pair; normalize off the critical path:
                        # reciprocal of rowsum row -> partition-broadcast via a
                        # DRAM bounce (step-0 partition APs are DRAM-only) ->
                        # one multiply into the pair-stacked e-major tile
                        osb = wk2.tile([65, QB], F32, name=f"osb{h2}",
                                       tag=f"osb{h2}", bufs=2)
                        nc.vector.tensor_copy(osb, ou[h2])
                        rr = wk2.tile([65, QB], F32, name="rr", tag="rr", bufs=4)
                        nc.vector.reciprocal_approx_fast(rr, osb)
                        rrd = dpool.tile([1, QB], F32, name="rrd", tag="rrd")
                        nc.gpsimd.dma_start(rrd, rr[64:65, :])
                        rb = wk2.tile([64, QB], F32, name="rb", tag="rb", bufs=4)
                        rr_bcast = bass.AP(tensor=rrd.tensor, offset=rrd.offset,
                                           ap=[[0, 64]] + list(rrd.ap[1:]))
                        nc.gpsimd.dma_start(rb, rr_bcast)
                        nc.vector.tensor_mul(stk[p][h2 * 64:(h2 + 1) * 64, :],
                                             osb[0:64, :], rb)
                # final projection for this q block
                for qt in range(NQT):
                    tsl = slice(qt * 128, (qt + 1) * 128)
                    ysb = wk2.tile([128, D], F32, name="ysb", tag="ysb", bufs=2)
                    for dc2 in range(2):
                        dsl = slice(dc2 * 512, (dc2 + 1) * 512)
                        yp = py.tile([128, 512], F32, name="yp", tag="y")
                        for p in range(NPAIR):
                            nc.tensor.matmul(yp, stk[p][:, tsl], wo_sb[:, p, dsl],
                                             start=(p == 0), stop=(p == NPAIR - 1))
                        nc.vector.tensor_copy(ysb[:, dsl], yp)
                    nc.sync.dma_start(out[qb * QB + qt * 128:
                                          qb * QB + (qt + 1) * 128, :], ysb)

    nc.compile()
    return nc


def _bf16(a):
    return np.ascontiguousarray(a.astype(ml_dtypes.bfloat16))


def _host_prep(inputs):
    x = np.asarray(inputs["x"], np.float32)
    enc = np.asarray(inputs["encoder_output"], np.float32)
    Wq = np.asarray(inputs["Wq"], np.float32)
    bq = np.asarray(inputs["bq"], np.float32)
    Wk = np.asarray(inputs["Wk"], np.float32)
    bk = np.asarray(inputs["bk"], np.float32)
    Wv = np.asarray(inputs["Wv"], np.float32)
    Wo = np.asarray(inputs["Wo"], np.float32)

    xt_b = [_bf16(x[b].T) for b in range(B)]
    et_b = [_bf16(enc[b].T) for b in range(B)]

    in_maps = []
    for c in range(NCORES):
        b = c // 4
        hb = HPC * (c % 4)

        wq_c = Wq[hb:hb + 4].reshape(2, 2, DC, 128, HD)  # [pair, hw, dc, dp, e]
        wq_c = wq_c.transpose(3, 0, 2, 1, 4).reshape(128, NPAIR, DC, 128)
        wk_c = Wk[hb:hb + 4].reshape(2, 2, DC, 128, HD)
        wk_c = wk_c.transpose(3, 0, 2, 1, 4).reshape(128, NPAIR, DC, 128)
        wv_c = Wv[hb:hb + 4].reshape(4, DC, 128, HD)
        wv_c = wv_c.transpose(2, 1, 0, 3).reshape(128, DC, 256)
        wo_c = Wo[hb * HD:(hb + 4) * HD].reshape(2, 2, HD, D)  # [pair, hw, e, d]
        wo_c = wo_c.transpose(1, 2, 0, 3).reshape(128, NPAIR, D)
        bq_c = bq[hb:hb + 4].reshape(2, 2, HD).transpose(1, 2, 0).reshape(128, NPAIR)
        bk_c = bk[hb:hb + 4].reshape(2, 2, HD).transpose(1, 2, 0).reshape(128, NPAIR)

        in_maps.append({
            "xt": xt_b[b],
            "et": et_b[b],
            "wq": _bf16(wq_c),
            "wk": _bf16(wk_c),
            "wv": _bf16(wv_c),
            "wo": _bf16(wo_c),
            "bq": np.ascontiguousarray(bq_c),
            "bk": np.ascontiguousarray(bk_c),
        })
    return in_maps


def kernel(**inputs):
    if "nc" not in _CACHE:
        _CACHE["nc"] = _build_program()
    nc = _CACHE["nc"]

    in_maps = _host_prep(inputs)
    res = None
    for attempt in range(3):
        try:
            res = run_bass_kernel_spmd(nc, in_maps, core_ids=list(range(NCORES)))
            break
        except Exception:
            if attempt == 2:
                raise
            import time
            time.sleep(5)
    _CACHE["last_results"] = res

    bv = np.asarray(inputs["bv"], np.float32)
    Wo = np.asarray(inputs["Wo"], np.float32)
    bo = np.asarray(inputs["bo"], np.float32)
    const_d = bo + np.einsum("he,hed->d", bv,
                             Wo.reshape(H, HD, D)).astype(np.float32)

    out = np.empty((B, S, D), np.float32)
    for b in range(B):
        acc = res.results[4 * b]["out"].astype(np.float32).copy()
        for c in range(4 * b + 1, 4 * b + 4):
            acc += res.results[c]["out"]
        out[b] = acc + const_d
    return out


# revision 26
# speedup vs baseline: 1.2573x; 1.0192x over previous
"""Trainium2 Bass kernel for CrossMultiHeadedSelfAttention.

Problem: B=2, SQ=SK=2048, D=1024, H=16, HD=64 cross-attention
  q = x @ Wq + bq ; k = enc @ Wk + bk ; v = enc @ Wv + bv   (per head)
  out = softmax(q k^T / sqrt(HD)) v  -> concat heads -> @ Wo + bo

Sharding: 8 cores = 2 batches x 4 head-groups (4 heads per core).
Each core computes a partial output projection over its 4 heads; the host
sums the 4 partials per batch and adds the constant term
(bo + sum_h bv_h @ Wo_h, exact because softmax rows sum to 1).

Device-side math (per core, bf16 matmuls, f32 accumulation):
  - x/enc are pre-transposed AND pre-cast to bf16 on the host, so xT/encT
    d-major tiles load with fully contiguous DMA
  - qT/kT in [head-pair e (128) x seq] layout, bias via per-partition
    tensor_scalar add; v in natural [s, 4*65] layout with a ones column
    per head (gives softmax row-sums for free in the attn@v matmul)
  - scoresT chunk = kT_h.T @ qT_h  -> exp (scale=1/8, no max subtraction:
    scores ~ N(0,1), |s|max ~ 6 so exp is safe in f32/bf16)
  - outU = v'_h.T @ expT  ([65 x 512] in PSUM, row 64 = softmax row-sum)
  - normalize without any transpose: reciprocal of row 64 -> tiny
    partition-broadcast DMA [1,512]->[64,512] -> one tensor_mul writes the
    normalized e-major tile into the pair-stacked stk buffer
  - y = sum_pairs stk_pair.T @ Wo_pair  (K=128), DMA partial to DRAM
"""

import sys

for _p in ("/opt/trn_rl_repo", "/root/.axon_site/_ro/trn_rl_repo"):
    if _p not in sys.path:
        sys.path.insert(0, _p)

import numpy as np
import ml_dtypes

import concourse.bass as bass
import concourse.tile as tile
from concourse import bacc, mybir
from concourse.bass_utils import run_bass_kernel_spmd

BF16 = mybir.dt.bfloat16
F32 = mybir.dt.float32
AF = mybir.ActivationFunctionType

B, S, D, H, HD = 2, 2048, 1024, 16, 64
NCORES = 8
HPC = 4          # heads per core
NPAIR = 2        # head pairs per core
DC = D // 128    # 8 d-chunks
KC = S // 128    # 16 k-chunks
NQB = 4          # q blocks of 512
QB = 512
NQT = QB // 128  # q tiles per block

_CACHE = {}


def _build_program():
    nc = bacc.Bacc("TRN2", target_bir_lowering=False, debug=False, num_devices=NCORES)

    xt = nc.dram_tensor("xt", [D, S], BF16, kind="ExternalInput").ap()
    et = nc.dram_tensor("et", [D, S], BF16, kind="ExternalInput").ap()
    wq = nc.dram_tensor("wq", [128, NPAIR, DC, 128], BF16, kind="ExternalInput").ap()
    wk = nc.dram_tensor("wk", [128, NPAIR, DC, 128], BF16, kind="ExternalInput").ap()
    wv = nc.dram_tensor("wv", [128, DC, 256], BF16, kind="ExternalInput").ap()
    wo = nc.dram_tensor("wo", [128, NPAIR, D], BF16, kind="ExternalInput").ap()
    bq = nc.dram_tensor("bq", [128, NPAIR], F32, kind="ExternalInput").ap()
    bk = nc.dram_tensor("bk", [128, NPAIR], F32, kind="ExternalInput").ap()
    out = nc.dram_tensor("out", [S, D], F32, kind="ExternalOutput").ap()

    with tile.TileContext(nc) as tc:
        from contextlib import ExitStack

        with ExitStack() as ctx:
            wts = ctx.enter_context(tc.tile_pool(name="wts", bufs=1))
            big = ctx.enter_context(tc.tile_pool(name="big", bufs=1))

            # weights via gpsimd SWDGE; bulk activations via sync HWDGE
            wq_sb = wts.tile([128, NPAIR, DC, 128], BF16, name="wq_sb")
            wk_sb = wts.tile([128, NPAIR, DC, 128], BF16, name="wk_sb")
            wv_sb = wts.tile([128, DC, 256], BF16, name="wv_sb")
            wo_sb = wts.tile([128, NPAIR, D], BF16, name="wo_sb")
            bq_sb = wts.tile([128, NPAIR], F32, name="bq_sb")
            bk_sb = wts.tile([128, NPAIR], F32, name="bk_sb")
            for sb, dr in ((wq_sb, wq), (wk_sb, wk), (wv_sb, wv), (wo_sb, wo),
                           (bq_sb, bq), (bk_sb, bk)):
                nc.gpsimd.dma_start(sb, dr)

            # column-block loads so the first projection chunk only waits on
            # ~1MB of activations, not the full 8MB
            xT = [big.tile([128, S], BF16, name=f"xT{d}") for d in range(DC)]
            eT = [big.tile([128, S], BF16, name=f"eT{d}") for d in range(DC)]
            for sb4 in range(NQB):
                sl = slice(sb4 * QB, (sb4 + 1) * QB)
                for d in range(DC):
                    nc.sync.dma_start(eT[d][:, sl], et[d * 128:(d + 1) * 128, sl])
                if sb4 == 0:
                    for d in range(DC):
                        nc.sync.dma_start(xT[d][:, sl],
                                          xt[d * 128:(d + 1) * 128, sl])
            for sb4 in range(1, NQB):
                sl = slice(sb4 * QB, (sb4 + 1) * QB)
                for d in range(DC):
                    nc.sync.dma_start(xT[d][:, sl], xt[d * 128:(d + 1) * 128, sl])

            # ---- unified PSUM pools (8 banks total, live for whole kernel) ----
            dpool = ctx.enter_context(tc.tile_pool(name="dpool", bufs=4,
                                                   space="DRAM"))
            psc = ctx.enter_context(tc.tile_pool(name="psc", bufs=2, space="PSUM"))
            pou = ctx.enter_context(tc.tile_pool(name="pou", bufs=2, space="PSUM"))
            py = ctx.enter_context(tc.tile_pool(name="py", bufs=2, space="PSUM"))
            wk2 = ctx.enter_context(tc.tile_pool(name="wk2", bufs=2))
            expp = ctx.enter_context(tc.tile_pool(name="expp", bufs=6))

            # ---- projections; only kT[0] + v gate the first attention ----
            qT = [big.tile([128, S], BF16, name=f"qT{p}") for p in range(NPAIR)]
            kT = [big.tile([128, S], BF16, name=f"kT{p}") for p in range(NPAIR)]
            v = [big.tile([128, HPC, 65], BF16, name=f"v{s}") for s in range(KC)]

            def project_k_chunk(p, sb4):
                # deferred projections use the y-slot (idle during attention)
                sl = slice(sb4 * QB, (sb4 + 1) * QB)
                pk = py.tile([128, QB], F32, name="pk", tag="y")
                for d in range(DC):
                    nc.tensor.matmul(pk, wk_sb[:, p, d, :], eT[d][:, sl],
                                     start=(d == 0), stop=(d == DC - 1))
                nc.vector.tensor_scalar_add(kT[p][:, sl], pk, bk_sb[:, p:p + 1])

            def project_q(p, qb, tag="sc"):
                qsl = slice(qb * QB, (qb + 1) * QB)
                pool = psc if tag == "sc" else py
                pq = pool.tile([128, QB], F32, name="pq", tag=tag)
                for d in range(DC):
                    nc.tensor.matmul(pq, wq_sb[:, p, d, :], xT[d][:, qsl],
                                     start=(d == 0), stop=(d == DC - 1))
                nc.vector.tensor_scalar_add(qT[p][:, qsl], pq, bq_sb[:, p:p + 1])

            def project_v_tile(s):
                pv = py.tile([128, 256], F32, name="pv", tag="y")
                for d in range(DC):
                    nc.tensor.matmul(pv, eT[d][:, s * 128:(s + 1) * 128],
                                     wv_sb[:, d, :],
                                     start=(d == 0), stop=(d == DC - 1))
                nc.vector.tensor_copy(
                    v[s][:, :, 0:64], pv.rearrange("p (h e) -> p h e", h=HPC))
                nc.vector.memset(v[s][:, :, 64:65], 1.0)

            # minimal prologue: just what (qb0, pair0, kc0..3) needs;
            # the rest of the projections are interjected into the first
            # q block's attention loops (PE slack there, ACT is the
            # bottleneck engine inside the kc loop)
            project_k_chunk(0, 0)
            for s in range(4):
                project_v_tile(s)
            project_q(0, 0)

            def interject_qb0_p0(kc):
                if kc == 0:
                    project_k_chunk(0, 1)
                elif kc == 1:
                    project_v_tile(4); project_v_tile(5)
                elif kc == 2:
                    project_v_tile(6); project_v_tile(7)
                elif kc == 4:
                    project_k_chunk(0, 2)
                elif kc == 5:
                    project_v_tile(8); project_v_tile(9)
                elif kc == 6:
                    project_v_tile(10); project_v_tile(11)
                elif kc == 8:
                    project_k_chunk(0, 3)
                elif kc == 9:
                    project_v_tile(12); project_v_tile(13)
                elif kc == 10:
                    project_v_tile(14); project_v_tile(15)
                elif kc == 12:
                    project_k_chunk(1, 0)
                elif kc == 13:
                    project_q(1, 0)
                elif kc == 15:
                    project_q(0, 1, tag="y")

            def interject_qb0_p1(kc):
                if kc == 0:
                    project_k_chunk(1, 1)
                elif kc == 4:
                    project_k_chunk(1, 2)
                elif kc == 8:
                    project_k_chunk(1, 3)
                elif kc == 11:
                    project_q(1, 1, tag="y")

            # ---- attention + output projection ----
            for qb in range(NQB):
                qsl = slice(qb * QB, (qb + 1) * QB)
                stk = [wk2.tile([128, QB], BF16, name=f"stk{p}", tag=f"stk{p}",
                                bufs=2) for p in range(NPAIR)]
                for p in range(NPAIR):
                    if qb == 0:
                        interject = interject_qb0_p0 if p == 0 else interject_qb0_p1
                    else:
                        # project the next block's qT for this pair during the
                        # ACT-bound kc loop, on the idle y slots
                        def interject(kc, p=p, qb=qb):
                            if kc == 11 and qb < NQB - 1:
                                project_q(p, qb + 1, tag="y")
                    ou = [pou.tile([65, QB], F32, name=f"ou{h2}", tag="ou")
                          for h2 in range(2)]
                    for kc in range(KC):
                        ksl = slice(kc * 128, (kc + 1) * 128)
                        sc = psc.tile([128, 2, QB], F32, name="sc", tag="sc")
                        ex = expp.tile([128, 2, QB], BF16, name="ex", tag="ex")
                        for h2 in range(2):
                            hp = slice(h2 * 64, (h2 + 1) * 64)
                            nc.tensor.matmul(sc[:, h2, :], kT[p][hp, ksl],
                                             qT[p][hp, qsl])
                        nc.scalar.activation(ex, sc, AF.Exp, scale=0.125)
                        for h2 in range(2):
                            nc.tensor.matmul(ou[h2], v[kc][:, 2 * p + h2, :],
                                             ex[:, h2, :],
                                             start=(kc == 0), stop=(kc == KC - 1))
                        if interject is not None:
                            interject(kc)
                    for h2 in range(2):
                        # copy PSUM->SBUF promptly so the ou slot frees for the
                        # next pair; normalize off the critical path:
                        # reciprocal of rowsum row -> partition-broadcast via a
                        # DRAM bounce (step-0 partition APs are DRAM-only) ->
                        # one multiply into the pair-stacked e-major tile
                        osb = wk2.tile([65, QB], F32, name=f"osb{h2}",
                                       tag=f"osb{h2}", bufs=2)
                        nc.vector.tensor_copy(osb, ou[h2])
                        rr = wk2.tile([65, QB], F32, name="rr", tag="rr", bufs=4)
                        nc.vector.reciprocal_approx_fast(rr, osb)
                        rrd = dpool.tile([1, QB], F32, name="rrd", tag="rrd")
                        nc.gpsimd.dma_start(rrd, rr[64:65, :])
                        rb = wk2.tile([64, QB], F32, name="rb", tag="rb", bufs=4)
                        rr_bcast = bass.AP(tensor=rrd.tensor, offset=rrd.offset,
                                           ap=[[0, 64]] + list(rrd.ap[1:]))
                        nc.gpsimd.dma_start(rb, rr_bcast)
                        nc.vector.tensor_mul(stk[p][h2 * 64:(h2 + 1) * 64, :],
                                             osb[0:64, :], rb)
                # final projection for this q block
                for qt in range(NQT):
                    tsl = slice(qt * 128, (qt + 1) * 128)
                    ysb = wk2.tile([128, D], F32, name="ysb", tag="ysb", bufs=2)
                    for dc2 in range(2):
                        dsl = slice(dc2 * 512, (dc2 + 1) * 512)
                        yp = py.tile([128, 512], F32, name="yp", tag="y")
                        for p in range(NPAIR):
                            nc.tensor.matmul(yp, stk[p][:, tsl], wo_sb[:, p, dsl],
                                             start=(p == 0), stop=(p == NPAIR - 1))
                        nc.vector.tensor_copy(ysb[:, dsl], yp)
                    nc.sync.dma_start(out[qb * QB + qt * 128:
                                          qb * QB + (qt + 1) * 128, :], ysb)

    nc.compile()
    return nc


def _bf16(a):
    return np.ascontiguousarray(a.astype(ml_dtypes.bfloat16))


def _host_prep(inputs):
    x = np.asarray(inputs["x"], np.float32)
    enc = np.asarray(inputs["encoder_output"], np.float32)
    Wq = np.asarray(inputs["Wq"], np.float32)
    bq = np.asarray(inputs["bq"], np.float32)
    Wk = np.asarray(inputs["Wk"], np.float32)
    bk = np.asarray(inputs["bk"], np.float32)
    Wv = np.asarray(inputs["Wv"], np.float32)
    Wo = np.asarray(inputs["Wo"], np.float32)

    xt_b = [_bf16(x[b].T) for b in range(B)]
    et_b = [_bf16(enc[b].T) for b in range(B)]

    in_maps = []
    for c in range(NCORES):
        b = c // 4
        hb = HPC * (c % 4)

        wq_c = Wq[hb:hb + 4].reshape(2, 2, DC, 128, HD)  # [pair, hw, dc, dp, e]
        wq_c = wq_c.transpose(3, 0, 2, 1, 4).reshape(128, NPAIR, DC, 128)
        wk_c = Wk[hb:hb + 4].reshape(2, 2, DC, 128, HD)
        wk_c = wk_c.transpose(3, 0, 2, 1, 4).reshape(128, NPAIR, DC, 128)
        wv_c = Wv[hb:hb + 4].reshape(4, DC, 128, HD)
        wv_c = wv_c.transpose(2, 1, 0, 3).reshape(128, DC, 256)
        wo_c = Wo[hb * HD:(hb + 4) * HD].reshape(2, 2, HD, D)  # [pair, hw, e, d]
        wo_c = wo_c.transpose(1, 2, 0, 3).reshape(128, NPAIR, D)
        bq_c = bq[hb:hb + 4].reshape(2, 2, HD).transpose(1, 2, 0).reshape(128, NPAIR)
        bk_c = bk[hb:hb + 4].reshape(2, 2, HD).transpose(1, 2, 0).reshape(128, NPAIR)

        in_maps.append({
            "xt": xt_b[b],
            "et": et_b[b],
            "wq": _bf16(wq_c),
            "wk": _bf16(wk_c),
            "wv": _bf16(wv_c),
            "wo": _bf16(wo_c),
            "bq": np.ascontiguousarray(bq_c),
            "bk": np.ascontiguousarray(bk_c),
        })
    return in_maps


def kernel(**inputs):
    if "nc" not in _CACHE:
        _CACHE["nc"] = _build_program()
    nc = _CACHE["nc"]

    in_maps = _host_prep(inputs)
    res = None
    for attempt in range(3):
        try:
            res = run_bass_kernel_spmd(nc, in_maps, core_ids=list(range(NCORES)))
            break
        except Exception:
            if attempt == 2:
                raise
            import time
            time.sleep(5)
    _CACHE["last_results"] = res

    bv = np.asarray(inputs["bv"], np.float32)
    Wo = np.asarray(inputs["Wo"], np.float32)
    bo = np.asarray(inputs["bo"], np.float32)
    const_d = bo + np.einsum("he,hed->d", bv,
                             Wo.reshape(H, HD, D)).astype(np.float32)

    out = np.empty((B, S, D), np.float32)
    for b in range(B):
        acc = res.results[4 * b]["out"].astype(np.float32).copy()
        for c in range(4 * b + 1, 4 * b + 4):
            acc += res.results[c]["out"]
        out[b] = acc + const_d
    return out
